# revision 12
# baseline (speedup 1.0000x reference)
"""Trainium2 Bass kernel: 3-layer GCN (GCNConv x3, relu, sigmoid) on 8 NeuronCores.

Strategy (dst-node partitioning, per sharding hint):
  - Host: add self-loops, compute symmetric norm, permute nodes into 128-slot
    tiles balanced by in-degree (snake pack), shard tiles across 8 cores.
    Per-edge norm coefficient c_e = w_e * deg^-1/2[dst]; the deg^-1/2[src]
    factor is folded into the gathered table rows.
  - Device, per layer k: each core computes its shard of the "table"
    H''_k = (dis * h_k) @ W_k (fp16), AllGather -> full table in every core's
    DRAM; dma_gather 512B rows for its edges; segment-sum via one-hot selector
    matmuls (c_e folded into the selector) accumulated in PSUM; evac =
    bias-add + Relu(scale=dis) -> next h''. Layer 3 (D_out=1) gathers scalars
    with indirect DMA and finishes with sigmoid(+b3).
  - Selector matrices / index streams are host-precomputed inputs (pure graph
    preprocessing); all matmul/gather/activation FLOPs run on device.
"""

import math
import os
from contextlib import ExitStack

import numpy as np

import concourse.bass as bass
import concourse.bacc as bacc
import concourse.mybir as mybir
import concourse.tile as tile
from concourse.bass import IndirectOffsetOnAxis
from concourse.bass_utils import run_bass_kernel_spmd
from concourse.masks import make_identity

F32 = mybir.dt.float32
F16 = mybir.dt.float16
I16 = mybir.dt.int16
I32 = mybir.dt.int32
P = 128

_LAST = {}


class Plan:
    """Host-side graph preprocessing: permutation, tiling, streams."""

    def __init__(self, edge_index, edge_weight, N, D, NC=8, G=4, HALF=32768):
        edge_index = np.asarray(edge_index)
        w = np.asarray(edge_weight, dtype=np.float32)
        src = edge_index[0].astype(np.int64)
        dst = edge_index[1].astype(np.int64)
        self.N, self.D, self.NC, self.HALF = N, D, NC, HALF

        tiles_pc = math.ceil(N / (NC * P))
        TILES = NC * tiles_pc
        SLOTS = TILES * P
        SPC = tiles_pc * P
        self.tiles_pc, self.TILES, self.SLOTS, self.SPC = tiles_pc, TILES, SLOTS, SPC

        # symmetric norm (self-loop weight 1)
        degw = np.bincount(dst, weights=w.astype(np.float64), minlength=N) + 1.0
        dis = (1.0 / np.sqrt(degw)).astype(np.float32)
        self.dis = dis
        indeg = np.bincount(dst, minlength=N) + 1

        # snake pack nodes into tiles by in-degree
        order = np.argsort(-indeg, kind="stable")
        ii = np.arange(N)
        rnd, pos = ii // TILES, ii % TILES
        tile_sorted = np.where(rnd % 2 == 0, pos, TILES - 1 - pos)
        perm = np.empty(N, np.int64)
        perm[order] = tile_sorted * P + rnd
        assert perm.max() < SLOTS
        self.perm = perm

        # edges + self loops, mapped to permuted ids
        es = np.concatenate([src, np.arange(N)])
        ed = np.concatenate([dst, np.arange(N)])
        ew = np.concatenate([w, np.ones(N, np.float32)])
        gs = perm[es]
        gd = perm[ed]
        c = (ew * dis[ed]).astype(np.float32)
        tl = gd // P
        sl = gd % P
        isB = (gs >= HALF).astype(np.int64)

        skey = tl * 2 + isB
        eord = np.argsort(skey, kind="stable")
        tl, sl, gs, c, isB, skey = (
            a[eord] for a in (tl, sl, gs, c, isB, skey)
        )
        cnt = np.bincount(skey, minlength=TILES * 2)
        cntA, cntB = cnt[0::2], cnt[1::2]
        K_A = max(1, math.ceil(cntA.max() / P))
        K_B = math.ceil(cntB.max() / P) if cntB.max() > 0 else 0
        self.K_A, self.K_B = K_A, K_B
        KT = K_A + K_B
        self.KT = KT

        starts = np.zeros(TILES * 2, np.int64)
        starts[1:] = np.cumsum(cnt)[:-1]
        rank = np.arange(len(tl)) - starts[skey]
        chunk_local = rank // P
        part = rank % P

        # group (of tiles) structure within a core
        G = min(G, tiles_pc)
        self.G = G
        n_groups = math.ceil(tiles_pc / G)
        gsizes = [min(G, tiles_pc - g * G) for g in range(n_groups)]
        cbs = np.concatenate([[0], np.cumsum([gs_ * KT for gs_ in gsizes])]).astype(
            np.int64
        )
        self.n_groups, self.gsizes, self.cbs = n_groups, gsizes, cbs
        TOTCH = tiles_pc * KT
        self.TOTCH = TOTCH

        core = tl // tiles_pc
        tic = tl % tiles_pc
        gi = tic // G
        tau = tic % G
        gA = np.array([gs_ * K_A for gs_ in gsizes])[gi]
        ch = np.where(
            isB == 0,
            cbs[gi] + tau * K_A + chunk_local,
            cbs[gi] + gA + tau * K_B + chunk_local,
        )
        assert (chunk_local < np.where(isB == 0, K_A, max(K_B, 1))).all()

        # selector stream [128, TOTCH, 128] fp16 per core (c folded in)
        snp = np.zeros((NC, P, TOTCH, P), np.float16)
        snp[core, part, ch, sl] = c.astype(np.float16)
        self.snp = snp

        # gather idx values per (chunk, part); pads stay 0
        idxv = np.zeros((NC, TOTCH, P), np.int32)
        idxv[core, ch, part] = np.where(isB == 1, gs - HALF, gs)
        # wrapped int16 stream: per group, A-call then B-call segments
        oA = np.zeros(n_groups, np.int64)
        oB = np.zeros(n_groups, np.int64)
        segs = [[] for _ in range(NC)]
        off = 0
        for g in range(n_groups):
            nchA = gsizes[g] * K_A
            nchB = gsizes[g] * K_B
            oA[g] = off
            off += nchA * 8
            oB[g] = off
            off += nchB * 8
            for cc in range(NC):
                a = idxv[cc, cbs[g] : cbs[g] + nchA].reshape(-1)
                segs[cc].append(a.reshape(-1, 16).T)
                if nchB:
                    b = idxv[cc, cbs[g] + nchA : cbs[g] + nchA + nchB].reshape(-1)
                    segs[cc].append(b.reshape(-1, 16).T)
        self.oA, self.oB = oA, oB
        self.NIX = off
        ix16 = np.zeros((NC, P, off), np.int16)
        for cc in range(NC):
            cat = np.concatenate(segs[cc], axis=1)
            assert cat.shape == (16, off)
            assert cat.min() >= 0 and cat.max() < 32768
            ix16[cc] = np.tile(cat.astype(np.int16), (8, 1))
        self.ix16 = ix16

        # per-slot dis in [slot, tile] layout per core
        dis_all = np.zeros(SLOTS, np.float32)
        dis_all[perm] = dis
        self.dist = (
            dis_all.reshape(NC, tiles_pc, P).transpose(0, 2, 1).copy()
        )  # [NC, 128, tiles_pc]

        self.meta = (
            N, D, NC, tiles_pc, SLOTS, SPC, K_A, K_B, G, n_groups,
            tuple(gsizes), tuple(int(v) for v in cbs), tuple(int(v) for v in oA),
            tuple(int(v) for v in oB), int(self.NIX), TOTCH, HALF,
        )

    def in_maps(self, x, W1, b1, W2, b2, W3, b3):
        N, D, NC, SLOTS, SPC = self.N, self.D, self.NC, self.SLOTS, self.SPC
        xs_all = np.zeros((SLOTS, D), np.float32)
        xs_all[self.perm] = np.asarray(x, np.float32)
        maps = []
        for cc in range(NC):
            maps.append(
                {
                    "xs": np.ascontiguousarray(xs_all[cc * SPC : (cc + 1) * SPC]),
                    "dist": np.ascontiguousarray(self.dist[cc]),
                    "w1": np.asarray(W1, np.float32),
                    "w2": np.asarray(W2, np.float32),
                    "w3": np.asarray(W3, np.float32).reshape(D, 1),
                    "b1r": np.tile(np.asarray(b1, np.float32)[None, :], (P, 1)),
                    "b2r": np.tile(np.asarray(b2, np.float32)[None, :], (P, 1)),
                    "b3r": np.full((P, 1), float(np.asarray(b3).reshape(-1)[0]), np.float32),
                    "ix16": self.ix16[cc],
                    "snp": self.snp[cc],
                }
            )
        return maps


def build_program(meta, stages=99):
    (N, D, NC, tiles_pc, SLOTS, SPC, K_A, K_B, G, n_groups, gsizes, cbs, oA, oB,
     NIX, TOTCH, HALF) = meta
    KT = K_A + K_B
    HALFR = min(SLOTS, HALF)

    nc = bacc.Bacc("TRN2", target_bir_lowering=False, num_devices=NC)
    xs = nc.declare_dram_parameter("xs", [SPC, D], F32, isOutput=False)
    dist = nc.declare_dram_parameter("dist", [P, tiles_pc], F32, isOutput=False)
    w1 = nc.declare_dram_parameter("w1", [D, D], F32, isOutput=False)
    w2 = nc.declare_dram_parameter("w2", [D, D], F32, isOutput=False)
    w3 = nc.declare_dram_parameter("w3", [D, 1], F32, isOutput=False)
    b1r = nc.declare_dram_parameter("b1r", [P, D], F32, isOutput=False)
    b2r = nc.declare_dram_parameter("b2r", [P, D], F32, isOutput=False)
    b3r = nc.declare_dram_parameter("b3r", [P, 1], F32, isOutput=False)
    ix16 = nc.declare_dram_parameter("ix16", [P, NIX], I16, isOutput=False)
    snp = nc.declare_dram_parameter("snp", [P, TOTCH, P], F16, isOutput=False)
    out = nc.declare_dram_parameter("out", [SPC, 1], F32, isOutput=True)

    shard1 = nc.dram_tensor("shard1", [SPC, D], F16)
    shard2 = nc.dram_tensor("shard2", [SPC, D], F16)
    shard3 = nc.dram_tensor("shard3", [SPC, P], F16)
    table1 = nc.dram_tensor("table1", [SLOTS, D], F16, addr_space="Shared")
    table2 = nc.dram_tensor("table2", [SLOTS, D], F16, addr_space="Shared")
    table3 = nc.dram_tensor("table3", [SLOTS, P], F16, addr_space="Shared")
    rg = [list(range(NC))]

    with tile.TileContext(nc) as tc, ExitStack() as ctx:
        const = ctx.enter_context(tc.tile_pool(name="const", bufs=1))
        xp = ctx.enter_context(tc.tile_pool(name="xp", bufs=3))
        xcp = ctx.enter_context(tc.tile_pool(name="xcp", bufs=3))
        htp = ctx.enter_context(tc.tile_pool(name="htp", bufs=3))
        shp = ctx.enter_context(tc.tile_pool(name="shp", bufs=3))
        tmpp = ctx.enter_context(tc.tile_pool(name="tmpp", bufs=3))
        gp = ctx.enter_context(tc.tile_pool(name="gp", bufs=2))
        sp = ctx.enter_context(tc.tile_pool(name="sp", bufs=2))
        ixp = ctx.enter_context(tc.tile_pool(name="ixp", bufs=2))
        ixbp = ctx.enter_context(tc.tile_pool(name="ixbp", bufs=2))
        osbp = ctx.enter_context(tc.tile_pool(name="osbp", bufs=3))
        pagg = ctx.enter_context(tc.tile_pool(name="pagg", bufs=4, space="PSUM"))
        paux = ctx.enter_context(tc.tile_pool(name="paux", bufs=2, space="PSUM"))

        ident = const.tile([P, P], F16, tag="ident")
        make_identity(nc, ident[:])
        w1sb = const.tile([P, 2 * D], F16, tag="w1sb")
        w2sb = const.tile([P, 2 * D], F16, tag="w2sb")
        w3sb = const.tile([P, 2], F16, tag="w3sb")
        for k in range(2):
            nc.gpsimd.dma_start(out=w1sb[:, k * D : (k + 1) * D], in_=w1[k * P : (k + 1) * P, :])
            nc.gpsimd.dma_start(out=w2sb[:, k * D : (k + 1) * D], in_=w2[k * P : (k + 1) * P, :])
            nc.gpsimd.dma_start(out=w3sb[:, k : k + 1], in_=w3[k * P : (k + 1) * P, :])
        b1sb = const.tile([P, D], F32, tag="b1sb")
        b2sb = const.tile([P, D], F32, tag="b2sb")
        b3sb = const.tile([P, 1], F32, tag="b3sb")
        nc.sync.dma_start(out=b1sb[:], in_=b1r[:])
        nc.sync.dma_start(out=b2sb[:], in_=b2r[:])
        nc.sync.dma_start(out=b3sb[:], in_=b3r[:])
        dsb = const.tile([P, tiles_pc], F32, tag="dsb")
        nc.sync.dma_start(out=dsb[:], in_=dist[:])
        hsb = const.tile([P, tiles_pc * D], F16, tag="hsb")

        def dense_phase(layer, wsb, nout, out_shard, out_dt):
            for t in range(tiles_pc):
                if layer == 1:
                    xt = xp.tile([P, D], F32, tag="xt")
                    nc.sync.dma_start(out=xt[:], in_=xs[t * P : (t + 1) * P, :])
                    src_t = xcp.tile([P, D], F16, tag="xc")
                    nc.vector.tensor_scalar(
                        out=src_t[:], in0=xt[:], scalar1=dsb[:, t : t + 1],
                        scalar2=None, op0=mybir.AluOpType.mult,
                    )
                    src_ap = src_t[:]
                else:
                    src_ap = hsb[:, t * D : (t + 1) * D]
                ptr = paux.tile([P, D], F16, space="PSUM", tag="ptr")
                nc.tensor.transpose(ptr[:, 0:P], src_ap[:, 0:P], ident[:])
                nc.tensor.transpose(ptr[:, P : 2 * P], src_ap[:, P : 2 * P], ident[:])
                htT = htp.tile([P, D], F16, tag="htT")
                nc.vector.tensor_copy(htT[:], ptr[:])
                pd = paux.tile([P, D], F32, space="PSUM", tag="pd")
                for k in range(2):
                    nc.tensor.matmul(
                        pd[:, :nout],
                        lhsT=htT[:, k * P : (k + 1) * P],
                        rhs=wsb[:, k * nout : (k + 1) * nout],
                        start=(k == 0),
                        stop=(k == 1),
                    )
                if nout == 1:
                    sh1 = shp.tile([P, 1], F32, tag="sh1")
                    nc.vector.tensor_copy(sh1[:], pd[:, :1])
                    sh = shp.tile([P, P], out_dt, tag="sh3")
                    nc.vector.tensor_copy(sh[:], sh1[:].to_broadcast([P, P]))
                    nc.sync.dma_start(
                        out=out_shard[t * P : (t + 1) * P, :], in_=sh[:]
                    )
                else:
                    sh = shp.tile([P, D], out_dt, tag="sh")
                    nc.vector.tensor_copy(sh[:, :nout], pd[:, :nout])
                    nc.sync.dma_start(
                        out=out_shard[t * P : (t + 1) * P, :], in_=sh[:, :nout]
                    )

        def allgather(shard, table):
            nc.gpsimd.collective_compute(
                "AllGather",
                mybir.AluOpType.bypass,
                ins=[shard[:]],
                outs=[table[:]],
                replica_groups=rg,
            )

        def agg_phase(table, bsb):
            for g in range(n_groups):
                gsz = gsizes[g]
                nchA, nchB = gsz * K_A, gsz * K_B
                nch = nchA + nchB
                cb = cbs[g]
                gb = gp.tile([P, G * KT, D], F16, tag="gb")
                ixa = ixp.tile([P, G * K_A * 8], I16, tag="ixa")
                nc.sync.dma_start(
                    out=ixa[:, : nchA * 8], in_=ix16[:, oA[g] : oA[g] + nchA * 8]
                )
                nc.gpsimd.dma_gather(
                    out_ap=gb[:, 0:nchA, :],
                    in_ap=table[:HALFR, :],
                    idxs_ap=ixa[:, : nchA * 8],
                    num_idxs=nchA * P,
                    num_idxs_reg=nchA * P,
                    elem_size=D,
                    single_packet=False,
                )
                if nchB:
                    ixb = ixbp.tile([P, G * K_B * 8], I16, tag="ixb")
                    nc.sync.dma_start(
                        out=ixb[:, : nchB * 8], in_=ix16[:, oB[g] : oB[g] + nchB * 8]
                    )
                    nc.gpsimd.dma_gather(
                        out_ap=gb[:, nchA:nch, :],
                        in_ap=table[HALF:, :],
                        idxs_ap=ixb[:, : nchB * 8],
                        num_idxs=nchB * P,
                        num_idxs_reg=nchB * P,
                        elem_size=D,
                    single_packet=False,
                    )
                ssb = sp.tile([P, G * KT, P], F16, tag="ssb")
                nc.sync.dma_start(out=ssb[:, :nch, :], in_=snp[:, cb : cb + nch, :])
                for tau in range(gsz):
                    pt = pagg.tile([P, D], F32, space="PSUM", tag="pt")
                    for j in range(K_A):
                        ch = tau * K_A + j
                        nc.tensor.matmul(
                            pt[:], lhsT=ssb[:, ch, :], rhs=gb[:, ch, :],
                            start=(j == 0), stop=(j == K_A - 1 and K_B == 0),
                        )
                    for j in range(K_B):
                        ch = nchA + tau * K_B + j
                        nc.tensor.matmul(
                            pt[:], lhsT=ssb[:, ch, :], rhs=gb[:, ch, :],
                            start=False, stop=(j == K_B - 1),
                        )
                    t = g * G + tau
                    tmp = tmpp.tile([P, D], F32, tag="tmp")
                    nc.vector.tensor_tensor(
                        out=tmp[:], in0=pt[:], in1=bsb[:], op=mybir.AluOpType.add
                    )
                    nc.scalar.activation(
                        out=hsb[:, t * D : (t + 1) * D], in_=tmp[:],
                        func=mybir.ActivationFunctionType.Relu,
                        scale=dsb[:, t : t + 1],
                    )

        def l3_agg():
            for g in range(n_groups):
                gsz = gsizes[g]
                nchA, nchB = gsz * K_A, gsz * K_B
                nch = nchA + nchB
                cb = cbs[g]
                gb3 = gp.tile([P, G * KT, P], F16, tag="gb3")
                ixa = ixp.tile([P, G * K_A * 8], I16, tag="ixa")
                nc.sync.dma_start(
                    out=ixa[:, : nchA * 8], in_=ix16[:, oA[g] : oA[g] + nchA * 8]
                )
                nc.gpsimd.dma_gather(
                    out_ap=gb3[:, 0:nchA, :],
                    in_ap=table3[:HALFR, :],
                    idxs_ap=ixa[:, : nchA * 8],
                    num_idxs=nchA * P,
                    num_idxs_reg=nchA * P,
                    elem_size=P,
                    single_packet=False,
                )
                if nchB:
                    ixb = ixbp.tile([P, G * K_B * 8], I16, tag="ixb")
                    nc.sync.dma_start(
                        out=ixb[:, : nchB * 8], in_=ix16[:, oB[g] : oB[g] + nchB * 8]
                    )
                    nc.gpsimd.dma_gather(
                        out_ap=gb3[:, nchA:nch, :],
                        in_ap=table3[HALF:, :],
                        idxs_ap=ixb[:, : nchB * 8],
                        num_idxs=nchB * P,
                        num_idxs_reg=nchB * P,
                        elem_size=P,
                    single_packet=False,
                    )
                ssb = sp.tile([P, G * KT, P], F16, tag="ssb")
                nc.sync.dma_start(out=ssb[:, :nch, :], in_=snp[:, cb : cb + nch, :])
                for tau in range(gsz):
                    pt = pagg.tile([P, D], F32, space="PSUM", tag="pt")
                    for j in range(K_A):
                        ch = tau * K_A + j
                        nc.tensor.matmul(
                            pt[:, :1], lhsT=ssb[:, ch, :], rhs=gb3[:, ch, 0:1],
                            start=(j == 0), stop=(j == K_A - 1 and K_B == 0),
                        )
                    for j in range(K_B):
                        ch = nchA + tau * K_B + j
                        nc.tensor.matmul(
                            pt[:, :1], lhsT=ssb[:, ch, :], rhs=gb3[:, ch, 0:1],
                            start=False, stop=(j == K_B - 1),
                        )
                    t = g * G + tau
                    osb = osbp.tile([P, 1], F32, tag="osb")
                    nc.scalar.activation(
                        out=osb[:], in_=pt[:, :1],
                        func=mybir.ActivationFunctionType.Sigmoid,
                        bias=b3sb[:, :1], scale=1.0,
                    )
                    nc.sync.dma_start(out=out[t * P : (t + 1) * P, :], in_=osb[:])

        if stages >= 1:
            with nc.named_scope("dense1"):
                dense_phase(1, w1sb, D, shard1, F16)
        if stages >= 2:
            with nc.named_scope("ag1"):
                allgather(shard1, table1)
        if stages >= 3:
            with nc.named_scope("agg1"):
                agg_phase(table1, b1sb)
        if stages >= 4:
            with nc.named_scope("dense2"):
                dense_phase(2, w2sb, D, shard2, F16)
        if stages >= 5:
            with nc.named_scope("ag2"):
                allgather(shard2, table2)
        if stages >= 6:
            with nc.named_scope("agg2"):
                agg_phase(table2, b2sb)
        if stages >= 7:
            with nc.named_scope("dense3"):
                dense_phase(3, w3sb, 1, shard3, F16)
        if stages >= 8:
            with nc.named_scope("ag3"):
                allgather(shard3, table3)
        if stages >= 9:
            with nc.named_scope("agg3"):
                l3_agg()
        if stages < 9:
            for t in range(tiles_pc):
                z = osbp.tile([P, 1], F32, tag="osb")
                nc.vector.memset(z[:], 0.5)
                nc.sync.dma_start(out=out[t * P : (t + 1) * P, :], in_=z[:])

    nc.compile()
    return nc


_PROG_CACHE = {}


def _get_program(meta):
    if meta not in _PROG_CACHE:
        _PROG_CACHE[meta] = build_program(meta)
    return _PROG_CACHE[meta]


def kernel(**inputs):
    x = np.asarray(inputs["x"], np.float32)
    N, D = x.shape
    plan = Plan(inputs["edge_index"], inputs["edge_weight"], N, D)
    nc = _get_program(plan.meta)
    maps = plan.in_maps(
        x, inputs["W1"], inputs["b1"], inputs["W2"], inputs["b2"],
        inputs["W3"], inputs["b3"],
    )
    trace = bool(int(os.environ.get("GCN_TRACE", "0")))
    res = run_bass_kernel_spmd(
        nc, maps, list(range(plan.NC)), trace=trace,
        trace_cores=list(range(plan.NC)) if trace else None,
    )
    _LAST["res"] = res
    _LAST["plan"] = plan
    full = np.concatenate([res.results[i]["out"][:, 0] for i in range(plan.NC)])
    return full[plan.perm].reshape(N, 1).astype(np.float32)
